# revision 15
# baseline (speedup 1.0000x reference)
"""Causal self-attention + output projection + residual + LayerNorm on 8
Trainium2 NeuronCores.

Problem: B=4, S=2048, D=1024, H=16, dk=64 (fp32).

Sharding: core c = 2*b + g handles batch b with heads [8g, 8g+8) through the
attention; the pair (2b, 2b+1) exchanges normalized per-head context via TWO
fused pair-AllGathers (one per query half), after which BOTH cores of the
pair compute the full [2048, 1024] output projection + residual + LayerNorm
for batch b (the program must be identical across cores; the host keeps core
2b's copy).

Schedule (v2):
 - QKV projections in bf16 (PE cost identical to fp32r, half the DMA/SBUF).
 - Wave A (queries 0:1024) runs concurrently with x/W quarters 2,3; wave B
   afterwards. Per (pair, wave): scores^T -> exp -> ctx with the softmax
   denominator from a ones-column appended to V.
 - One AllGather per wave (not per head-pair): 15us fixed cost each in the
   comms model, issued from gpsimd; out-projection for the wave's 8 query
   tiles is emitted interleaved with the NEXT wave's compute so the tensor
   engine fills the activation-engine-bound stretches.
 - LayerNorm runs entirely off the Activation engine (which must keep the
   exp table loaded): bn_stats/bn_aggr + a Newton-iteration rsqrt on DVE,
   elementwise applies on Pool. Avoids exp<->sqrt act-table thrashing.
 - PSUM: one shared 3-slot rotation ("mm", 2 banks each) for QKV groups,
   score tiles and out-proj groups + 2 slots for ctx accumulators = 8 banks.
"""

import numpy as np
from contextlib import ExitStack

import concourse.bass as bass
import concourse.mybir as mybir
import concourse.tile as tile
import bass_rust
from concourse.tile import ScopedClock
from concourse.bass_utils import run_bass_kernel_spmd

FP = mybir.dt.float32
BF = mybir.dt.bfloat16
AF = mybir.ActivationFunctionType
ALU = mybir.AluOpType

B, S, D, H, DK = 4, 2048, 1024, 16, 64
N_CORES = 8
HPC = H // 2          # heads per core = 8
NEG = -1e9
EPS = 1e-6

# ---------------------------------------------------------------------------
# Compat shims: this walrus build rejects instructions with more than one
# sync-wait condition; split extra waits onto same-engine NoOp carriers.
# ---------------------------------------------------------------------------
_ws_ctr = [0]


def _split_waits_in_ordered(ordered):
    for bb_name, insts in list(ordered.items()):
        new = []
        for inst in insts:
            si = inst.sync_info
            if si is None:
                new.append(inst)
                continue
            waits = list(si.on_wait)
            if len(waits) > 1:
                head = len(waits) - 1
                for i in range(head):
                    _ws_ctr[0] += 1
                    carrier = mybir.InstNoOp(
                        name=f"I-ws{_ws_ctr[0]}", engine=inst.engine
                    )
                    carrier.sync_info = bass_rust.SyncInfo(
                        on_wait=[waits[i]], on_update=[]
                    )
                    new.append(carrier)
                inst.sync_info = bass_rust.SyncInfo(
                    on_wait=waits[head:], on_update=si.on_update
                )
            new.append(inst)
        ordered[bb_name] = new


_orig_lower = tile.TileContext._lower_ordered_insts


def _patched_lower(self, ordered):
    _split_waits_in_ordered(ordered)
    return _orig_lower(self, ordered)


def _split_drain_and_barrier(self, tick_clock, wait_clock):
    drain_inst = self.nc.sync.drain()
    wait_clock.add_sem_waits(
        drain_inst.ins, ScopedClock({None: tick_clock.global_clock})
    )
    si = drain_inst.ins.sync_info
    waits = list(si.on_wait)
    if len(waits) > 1:
        drain_inst.ins.sync_info = bass_rust.SyncInfo(
            on_wait=waits[:1], on_update=si.on_update
        )
        for i in range(1, len(waits)):
            d2 = self.nc.sync.drain()
            d2.ins.sync_info = bass_rust.SyncInfo(
                on_wait=[waits[i]], on_update=[]
            )
    self.nc.all_engine_barrier()
    assert self.sems is not None
    popped = self.nc._tile_sem_poison_stack.pop()
    assert popped is self._sem_poison
    self.nc.clear_and_free_semaphores(list(self.sems.allocated().values()))
    self.nc.all_engine_barrier()


def _install_compat():
    tile.TileContext._lower_ordered_insts = _patched_lower
    tile.TileContext._drain_and_barrier = _split_drain_and_barrier


# ---------------------------------------------------------------------------
# Program builder
# ---------------------------------------------------------------------------
_cached_nc = {}


def _ap(tensor, offset, dims):
    return bass.AP(tensor=tensor, offset=offset, ap=[list(d) for d in dims])


def _mm(nc, out, lhsT, rhs, **kw):
    nc.tensor.matmul(out, lhsT, rhs, **kw)


def build_nc(reps=1):
    if reps in _cached_nc:
        return _cached_nc[reps]
    _install_compat()
    nc = bass.Bass("TRN2", target_bir_lowering=False, debug=False,
                   num_devices=N_CORES)

    xT = nc.dram_tensor("xT", [D, S], BF, kind="ExternalInput")
    xres = nc.dram_tensor("xres", [S, D], FP, kind="ExternalInput")
    wq = nc.dram_tensor("wq", [D, 512], BF, kind="ExternalInput")
    wk = nc.dram_tensor("wk", [D, 512], BF, kind="ExternalInput")
    wv = nc.dram_tensor("wv", [D, 512], BF, kind="ExternalInput")
    bq = nc.dram_tensor("bq", [512], FP, kind="ExternalInput")
    bk = nc.dram_tensor("bk", [512], FP, kind="ExternalInput")
    bv = nc.dram_tensor("bv", [512], FP, kind="ExternalInput")
    wo = nc.dram_tensor("wo", [D, D], BF, kind="ExternalInput")
    gamma = nc.dram_tensor("gamma", [D], FP, kind="ExternalInput")
    beta = nc.dram_tensor("beta", [D], FP, kind="ExternalInput")
    mneg = nc.dram_tensor("mneg", [128, 128], FP, kind="ExternalInput")
    yout = nc.dram_tensor("y", [S, D], FP, kind="ExternalOutput")

    NKT = S // 128            # 16 k-tiles over the sequence

    io = (xT, xres, wq, wk, wv, bq, bk, bv, wo, gamma, beta, mneg, yout)
    with tile.TileContext(nc) as tc:
        with ExitStack() as ctx:
            dram = ctx.enter_context(
                tc.tile_pool(name="dram", bufs=1, space="DRAM"))
            for r in range(reps):
                _emit_body(nc, tc, ctx, dram, io, r, NKT)

    _cached_nc[reps] = nc
    return nc


def _emit_body(nc, tc, ctx, dram, io, r, NKT):
    (xT, xres, wq, wk, wv, bq, bk, bv, wo, gamma, beta, mneg, yout) = io

    # ---- long-lived pools (stack order: released in reverse at the end) ----
    pctx = tc.alloc_tile_pool(name=f"pctx{r}", bufs=1)
    ctxT = pctx.tile([128, 4, S], BF)      # normalized ctx^T, packed pairs
    wo_t = pctx.tile([128, 8, D], BF)
    gam = pctx.tile([128, D], FP)
    bet = pctx.tile([128, D], FP)
    mneg_t = pctx.tile([128, 128], FP)
    eps_t = pctx.tile([128, 1], FP)
    nc.vector.memset(eps_t, EPS)
    pln = tc.alloc_tile_pool(name=f"pln{r}", bufs=2)     # LN working tiles
    pu = tc.alloc_tile_pool(name=f"pu{r}", bufs=4)       # ctxU + bca
    p2e = tc.alloc_tile_pool(name=f"p2e{r}", bufs=4)     # expS
    pqkv = tc.alloc_tile_pool(name=f"pqkv{r}", bufs=1)
    QT = pqkv.tile([128, 4, S], BF)        # [hd%128, hd//128, s]
    KT = pqkv.tile([128, 4, S], BF)
    V = pqkv.tile([128, NKT, HPC, 65], BF)  # per-head V + ones column
    pmm = tc.alloc_tile_pool(name=f"pmm{r}", bufs=1, space="PSUM")

    nc.vector.memset(V[:, :, :, 64:65], 1.0)

    # ---- phase-1 pools (released after the last quarter) ----
    p1w = tc.alloc_tile_pool(name=f"p1w{r}", bufs=1)
    p1x = tc.alloc_tile_pool(name=f"p1x{r}", bufs=2)
    wq_t = p1w.tile([128, 8, 512], BF)
    wk_t = p1w.tile([128, 8, 512], BF)
    wv_t = p1w.tile([128, 8, 512], BF)
    bq_t = p1w.tile([128, 4], FP)
    bk_t = p1w.tile([128, 4], FP)
    # sync DMA queue order = priority: biases + chunked wq/wk first so the
    # first matmul only waits for wq chunk 0; wo/gamma/beta (needed ~140us)
    # trail behind.
    nc.sync.dma_start(out=bq_t, in_=_ap(bq, 0, [[1, 128], [128, 4]]))
    nc.sync.dma_start(out=bk_t, in_=_ap(bk, 0, [[1, 128], [128, 4]]))
    for c in range(2):
        nc.sync.dma_start(
            out=wq_t[:, 4 * c:4 * c + 4, :],
            in_=_ap(wq, 512 * 128 * 4 * c, [[512, 128], [512 * 128, 4], [1, 512]]))
    for c in range(2):
        nc.sync.dma_start(
            out=wk_t[:, 4 * c:4 * c + 4, :],
            in_=_ap(wk, 512 * 128 * 4 * c, [[512, 128], [512 * 128, 4], [1, 512]]))
    nc.gpsimd.dma_start(
        out=wv_t, in_=_ap(wv, 0, [[512, 128], [512 * 128, 8], [1, 512]]))
    bv_bc = p1w.tile([128, 8, 64], FP)
    nc.sync.dma_start(out=bv_bc, in_=_ap(bv, 0, [[0, 128], [64, 8], [1, 64]]))
    nc.sync.dma_start(out=mneg_t, in_=mneg[:, :])
    nc.sync.dma_start(
        out=wo_t, in_=_ap(wo, 0, [[D, 128], [128 * D, 8], [1, D]]))
    nc.sync.dma_start(out=gam, in_=_ap(gamma, 0, [[0, 128], [1, D]]))
    nc.sync.dma_start(out=bet, in_=_ap(beta, 0, [[0, 128], [1, D]]))

    def emit_quarter(sq):
        # two half-tiles of x^T (D-tiles 0-3 / 4-7) for cheaper double-buffer
        xqh = []
        for dh in range(2):
            xq = p1x.tile([128, 4, 512], BF, name=f"xq{dh}", tag="xq", bufs=3)
            nc.gpsimd.dma_start(
                out=xq, in_=_ap(xT, 512 * sq + dh * 4 * 128 * S,
                                [[S, 128], [128 * S, 4], [1, 512]]))
            xqh.append(xq)
        for wt, bt, dst in ((wq_t, bq_t, QT), (wk_t, bk_t, KT)):
            for mt in range(4):
                ps = pmm.tile([128, 2, 512], FP, name="ps1", tag="mm", bufs=3)
                for d in range(8):
                    _mm(nc, ps[:, 0, :], wt[:, d, 128 * mt:128 * (mt + 1)],
                        xqh[d // 4][:, d % 4, :],
                        start=(d == 0), stop=(d == 7))
                nc.vector.tensor_scalar_add(
                    dst[:, mt, 512 * sq:512 * (sq + 1)], ps[:, 0, :],
                    bt[:, mt:mt + 1])
        for st in range(4):
            ps = pmm.tile([128, 2, 512], FP, name="ps1v", tag="mm", bufs=3)
            for d in range(8):
                _mm(nc, ps[:, 0, :],
                    xqh[d // 4][:, d % 4, 128 * st:128 * (st + 1)],
                    wv_t[:, d, :], start=(d == 0), stop=(d == 7))
            sg = 4 * sq + st
            nc.vector.tensor_add(
                V[:, sg, :, 0:64],
                ps[:, 0, :].rearrange("p (h e) -> p h e", h=HPC), bv_bc)

    def emit_halfwave(p, w, jl):
        """scores+exp+ctx for (pair p, wave w, 512-query window jl), then
        normalize that window into ctxT and stage its agin slice. The
        normalize runs on Pool (the DMA-round-trip stall must not sit in
        the DVE stream, which has PSUM-freeing work queued behind it)."""
        qlo = 1024 * w
        jlo = qlo + 512 * jl
        klast = (jlo + 512) // 128 - 1
        ctxU = [pu.tile([65, 512], BF, name=f"cu{p}{w}{jl}{h}", tag="cu",
                        bufs=6) for h in range(2)]
        cps = [pmm.tile([65, 512], FP, name=f"cps{h}", tag="cps",
                        bufs=2) for h in range(2)]
        for k in range(klast + 1):
            clo = max(qlo, 128 * k)
            cstart = max(clo, jlo)
            blen = jlo + 512 - cstart
            doff = cstart - jlo
            s2 = pmm.tile([128, 2, 512], FP, name="s2", tag="mm", bufs=3)
            for h in range(2):
                rows = slice(64 * h, 64 * h + 64)
                _mm(nc, s2[:, h, 0:blen],
                    KT[rows, p, 128 * k:128 * (k + 1)],
                    QT[rows, p, cstart:cstart + blen],
                    start=True, stop=True, tile_position=(64 * h, 0))
            if cstart == 128 * k:
                dv = s2[:, :, 0:128]
                mb = _ap(mneg_t.tensor, mneg_t.offset,
                         [mneg_t.ap[0], [0, 2], mneg_t.ap[1]])
                nc.vector.tensor_add(dv, dv, mb)
            expS = p2e.tile([128, 2, 512], BF, name="expS", tag="expS")
            nc.scalar.activation(expS[:, :, 0:blen], s2[:, :, 0:blen],
                                 AF.Exp)
            for h in range(2):
                HH = 2 * p + h
                _mm(nc, cps[h][:, doff:doff + blen], V[:, k, HH, :],
                    expS[:, h, 0:blen],
                    start=(k == 0), stop=(k == klast))
        for h in range(2):
            nc.vector.tensor_copy(ctxU[h], cps[h])
        # normalize: ctxT rows = ctxU rows / denominator row (Pool divide)
        dend = dram.tile([2, 512], BF, name=f"dend{r}_{p}_{w}_{jl}")
        nc.sync.dma_start(out=dend[0:1, :], in_=ctxU[0][64:65, :])
        nc.sync.dma_start(out=dend[1:2, :], in_=ctxU[1][64:65, :])
        bca = pu.tile([64, 2, 512], BF, name="bca", tag="bca", bufs=3)
        for h in range(2):
            nc.sync.dma_start(
                out=bca[:, h, :],
                in_=_ap(dend.tensor, dend.offset + h * 512, [[0, 64], [1, 512]]))
        with nc.allow_low_precision(
                reason="softmax denominator ~2e3, bf16 rel err 4e-3 ok"):
            nc.vector.reciprocal(bca, bca)
        nc.gpsimd.tensor_mul(
            ctxT[0:64, p, jlo:jlo + 512], ctxU[0][0:64, :], bca[:, 0, :])
        nc.gpsimd.tensor_mul(
            ctxT[64:128, p, jlo:jlo + 512], ctxU[1][0:64, :], bca[:, 1, :])
        agi, acol = ("ag0", 512 * jl) if w == 0 else (
            ("ag1", 0) if jl == 0 else ("ag2", 0))
        agin = late[agi]
        nc.sync.dma_start(
            out=agin[128 * p:128 * (p + 1), acol:acol + 512],
            in_=ctxT[:, p, jlo:jlo + 512])

    late = {}

    def emit_ag(key, pw, width):
        """one fused pair-AllGather over `width` query columns."""
        agout = dram.tile([1024, width], BF, name=f"agout{r}_{key}")
        nc.gpsimd.collective_compute(
            "AllGather", mybir.AluOpType.bypass,
            replica_groups=[[0, 1], [2, 3], [4, 5], [6, 7]],
            ins=[late[key].opt()], outs=[agout.opt()])
        ctxW = []
        for kt in range(8):
            cw = pw.tile([128, width], BF, name=f"cw{r}_{key}_{kt}")
            eng = nc.gpsimd if kt % 2 == 0 else nc.sync
            eng.dma_start(out=cw, in_=agout[128 * kt:128 * (kt + 1), :])
            ctxW.append(cw)
        return ctxW

    def emit_outproj(st, ctxW, base_st, act_ln):
        """out-proj + residual + LayerNorm for sequence tile st (128 rows).

        act_ln=False (tiles overlapping exp waves): LayerNorm runs on
        DVE/Pool with a Newton-iteration rsqrt, keeping the Activation
        engine's exp table resident. act_ln=True (after the last exp):
        baseline Activation-engine chain (sqrt/identity/copy share one
        table, loaded once)."""
        stw = st - base_st
        xr = pln.tile([128, D], FP, name="xr", tag="xr", bufs=3)
        nc.sync.dma_start(out=xr, in_=xres[128 * st:128 * (st + 1), :])
        yt = pln.tile([128, D], FP, name="yt", tag="yt", bufs=3)
        for dsl in range(2):
            ps = pmm.tile([128, 2, 512], FP, name="ps3", tag="mm", bufs=3)
            for kt in range(8):
                _mm(nc, ps[:, 0, :], ctxW[kt][:, 128 * stw:128 * (stw + 1)],
                    wo_t[:, kt, 512 * dsl:512 * (dsl + 1)],
                    start=(kt == 0), stop=(kt == 7))
            nc.vector.tensor_add(
                yt[:, 512 * dsl:512 * (dsl + 1)], ps[:, 0, :],
                xr[:, 512 * dsl:512 * (dsl + 1)])
        stats = pln.tile([128, 2, 6], FP, name="stats", tag="stats", bufs=4)
        for hh in range(2):
            nc.vector.bn_stats(
                stats[:, hh, :], yt[:, 512 * hh:512 * (hh + 1)])
        mv = pln.tile([128, 2], FP, name="mv", tag="mv", bufs=4)
        nc.vector.bn_aggr(mv, stats)
        nmu = pln.tile([128, 1], FP, name="nmu", tag="nmu", bufs=4)
        nc.vector.tensor_scalar_mul(nmu, mv[:, 0:1], -1.0)
        rs = pln.tile([128, 1], FP, name="rs", tag="rs", bufs=4)
        if act_ln:
            sd = pln.tile([128, 1], FP, name="sd", tag="sd", bufs=4)
            nc.scalar.activation(sd, mv[:, 1:2], AF.Sqrt, bias=eps_t,
                                 scale=1.0)
            nc.vector.reciprocal(rs, sd)
            ut = pln.tile([128, D], FP, name="ut", tag="ut", bufs=3)
            nc.scalar.activation(ut, yt, AF.Identity, bias=nmu, scale=1.0)
            nc.scalar.activation(yt, ut, AF.Copy, bias=0.0, scale=rs)
        else:
            # rstd = 1/sqrt(var+eps): linear seed + 2 Newton steps on Pool
            # (var(y) ~ 1 + attention-output variance, well inside [0.75,2.5])
            vt = pln.tile([128, 1], FP, name="vt", tag="vt", bufs=4)
            nc.vector.tensor_scalar_add(vt, mv[:, 1:2], EPS)
            nc.gpsimd.tensor_scalar(rs, vt, -0.2984, 1.3205, ALU.mult, ALU.add)
            hh_t = pln.tile([128, 1], FP, name="hh", tag="hh", bufs=4)
            for _ in range(2):
                nc.gpsimd.tensor_mul(hh_t, rs, rs)
                nc.gpsimd.tensor_mul(hh_t, hh_t, vt)
                nc.gpsimd.tensor_scalar(hh_t, hh_t, -0.5, 1.5, ALU.mult, ALU.add)
                nc.gpsimd.tensor_mul(rs, rs, hh_t)
            nc.gpsimd.tensor_scalar(yt, yt, nmu, rs, ALU.add, ALU.mult)
        nc.gpsimd.tensor_mul(yt, yt, gam)
        nc.vector.tensor_add(yt, yt, bet)
        nc.sync.dma_start(
            out=yout[128 * st:128 * (st + 1), :], in_=yt)

    # ---------------- emission schedule ----------------
    late["ag0"] = dram.tile([512, 1024], BF, name=f"agin{r}_0")
    late["ag1"] = dram.tile([512, 512], BF, name=f"agin{r}_1")
    late["ag2"] = dram.tile([512, 512], BF, name=f"agin{r}_2")
    # quarters 2/3 interleave between wave-A pairs so their QT/KT psum-adds
    # don't queue on DVE behind all of wave A's copies (wave B p0 needs them)
    emit_quarter(0)
    emit_quarter(1)
    for p in (0, 1):
        emit_halfwave(p, 0, 0)
        emit_halfwave(p, 0, 1)
    emit_quarter(2)
    emit_halfwave(2, 0, 0)
    emit_halfwave(2, 0, 1)
    emit_quarter(3)
    emit_halfwave(3, 0, 0)
    emit_halfwave(3, 0, 1)
    p1x.release()
    p1w.release()

    pw = tc.alloc_tile_pool(name=f"pw{r}", bufs=1)
    ctxWA = emit_ag("ag0", pw, 1024)
    # out-proj tiles 0-3 interleave into wave B (held back so the PE stream
    # never head-of-line blocks on ctxWA); the wave-B AllGathers are split
    # per 512-query window: AG-B1 launches before pair 3's second window
    # even computes, and AG-B2 hides under out-proj tiles 4-11.
    emit_halfwave(0, 1, 0)
    emit_halfwave(0, 1, 1)
    emit_halfwave(1, 1, 0)
    emit_halfwave(1, 1, 1)
    emit_outproj(0, ctxWA, 0, False)
    emit_outproj(1, ctxWA, 0, False)
    emit_halfwave(2, 1, 0)
    emit_halfwave(2, 1, 1)
    emit_outproj(2, ctxWA, 0, False)
    emit_outproj(3, ctxWA, 0, False)
    emit_halfwave(3, 1, 0)
    ctxWB1 = emit_ag("ag1", pw, 512)
    emit_halfwave(3, 1, 1)
    ctxWB2 = emit_ag("ag2", pw, 512)
    for st in (4, 5, 6, 7):
        emit_outproj(st, ctxWA, 0, True)
    for st in range(8, 12):
        emit_outproj(st, ctxWB1, 8, True)
    for st in range(12, 16):
        emit_outproj(st, ctxWB2, 12, True)

    pw.release()
    pmm.release()
    pqkv.release()
    p2e.release()
    pu.release()
    pln.release()
    pctx.release()


# ---------------------------------------------------------------------------
# Host-side entry point
# ---------------------------------------------------------------------------
def make_in_maps(x, Wq, bq, Wk, bk, Wv, bv, Wo, bo, gamma, beta):
    import ml_dtypes
    bf16 = ml_dtypes.bfloat16
    x = np.asarray(x, np.float32)
    WqS = (np.asarray(Wq, np.float32) / np.sqrt(np.float32(DK))).reshape(D, H * DK)
    bqS = (np.asarray(bq, np.float32) / np.sqrt(np.float32(DK))).reshape(H * DK)
    WkF = np.asarray(Wk, np.float32).reshape(D, H * DK)
    bkF = np.asarray(bk, np.float32).reshape(H * DK)
    WvF = np.asarray(Wv, np.float32).reshape(D, H * DK)
    bvF = np.asarray(bv, np.float32).reshape(H * DK)
    WoF = np.ascontiguousarray(
        np.asarray(Wo, np.float32).reshape(H * DK, D).astype(bf16))
    boF = np.asarray(bo, np.float32)
    gF = np.ascontiguousarray(np.asarray(gamma, np.float32))
    btF = np.ascontiguousarray(np.asarray(beta, np.float32))
    kk = np.arange(128)[:, None]
    qq = np.arange(128)[None, :]
    mneg = np.where(kk <= qq, 0.0, NEG).astype(np.float32)

    in_maps = []
    for c in range(N_CORES):
        b, g = divmod(c, 2)
        cols = slice(512 * g, 512 * (g + 1))
        in_maps.append({
            "xT": np.ascontiguousarray(x[b].T.astype(bf16)),
            "xres": np.ascontiguousarray(x[b] + boF[None, :]),
            "wq": np.ascontiguousarray(WqS[:, cols].astype(bf16)),
            "wk": np.ascontiguousarray(WkF[:, cols].astype(bf16)),
            "wv": np.ascontiguousarray(WvF[:, cols].astype(bf16)),
            "bq": np.ascontiguousarray(bqS[cols]),
            "bk": np.ascontiguousarray(bkF[cols]),
            "bv": np.ascontiguousarray(bvF[cols]),
            "wo": WoF,
            "gamma": gF,
            "beta": btF,
            "mneg": mneg,
        })
    return in_maps


def kernel(x, Wq, bq, Wk, bk, Wv, bv, Wo, bo, gamma, beta):
    nc = build_nc()
    in_maps = make_in_maps(x, Wq, bq, Wk, bk, Wv, bv, Wo, bo, gamma, beta)
    r = run_bass_kernel_spmd(nc, in_maps, list(range(N_CORES)))
    out = np.empty((B, S, D), np.float32)
    for b in range(B):
        out[b] = r.results[2 * b]["y"]
    return out
